# revision 1
# baseline (speedup 1.0000x reference)
"""BatchAllTripletLoss on 8 Trainium2 NeuronCores (sparsity version).

Contract: kernel(**inputs) takes the FULL inputs (embs [512,128] f32,
idtys [512] int64) and returns the FULL output (scalar f32 loss).

Math: d = pairwise euclidean distances [512,512];
  loss = sum_{a,p,n} relu(d[a,p]-d[a,n]+margin)*mask / (num_pos + eps)
The triplet mask factorizes as pos[a,p]*neg[a,n] (pos: same id, p!=a;
neg: different id). With 64 ids over 512 samples, each anchor has only
~8 valid positives, so instead of brute-forcing all 512 p columns we
enumerate, per anchor, the members of its id group (ranked by a
device-side counting argsort) and only process those columns:

 1. d rows for this core's 128 anchors via PE matmul (+sq rank-1 folds),
    dneg = d + BIG*same  (neg mask folded; pushes relu/count to 0).
 2. Group member table: rank R_i = #(j<i with id_j==id_i) via a fused
    is_lt*same row-reduce; scatter index i into a DRAM table at row
    id_i*32 + perm(R_i) (indirect DMA). perm rotates ranks so that THIS
    core's parity class (R%2 == core parity) lands in columns 0..15 --
    per-core variation rides in input data, the program stays SPMD.
 3. Gather each anchor's member row [128,32] (indirect DMA by id), then
    per k-column: gather member embeddings [128,128], rowdot -> d[a,p]
    via sqrt(sqA+sqP-2dot), x = (d+margin)*(valid & p!=a).
 4. Main loop over just 16 k-columns (vs 512 brute-force):
    ACT: t = relu(x - y) bf16; DVE: g = 1[y_bf16 < x] bf16; the PE
    reduces every tile with ones[128,1] matmuls accumulated into PSUM.
Per-core output [1,2] = (relu sum, count); host sums cores and divides.
"""

import numpy as np

B = 512
D = 128
NCORES = 8
AH = 128          # anchors per core
KMAX = 16         # member-table width (max group size supported)
KP = 8            # k-columns processed per core (rank-half split of KMAX)
MARGIN = 0.2
BIG = 1.0e6

_CACHE = {}


def _build_bass():
    import concourse.bass as bass
    import concourse.tile as tile
    from concourse import mybir

    f32 = mybir.dt.float32
    i32 = mybir.dt.int32
    bf16 = mybir.dt.bfloat16
    AF = mybir.ActivationFunctionType
    OP = mybir.AluOpType
    IOA = bass.IndirectOffsetOnAxis
    X = mybir.AxisListType.X

    nc = bass.Bass()

    emT = nc.dram_tensor("emT", [D, B], f32, kind="ExternalInput")     # embs.T
    emTA = nc.dram_tensor("emTA", [D, AH], f32, kind="ExternalInput")  # anchor cols
    rows = nc.dram_tensor("rows", [1, 2 * B], f32, kind="ExternalInput")  # [ids|idx]
    # cols = [idsA, idxA, idsAll(4), idxAll(4), kidx(KP), parc, rowb, gcol]
    cols = nc.dram_tensor("cols", [AH, 13 + KP], f32, kind="ExternalInput")
    idsAr = nc.dram_tensor("idsAr", [1, AH], f32, kind="ExternalInput")  # ids of anchors, row
    out = nc.dram_tensor("out", [1, 2], f32, kind="ExternalOutput")

    dchd = nc.dram_tensor("dchd", [AH * B, 1], f32)      # d rows staged for gather

    with tile.TileContext(nc) as tc:
        with (
            tc.tile_pool(name="sb", bufs=1) as sb,
            tc.tile_pool(name="psrow", bufs=1, space="PSUM") as psrow,
            tc.tile_pool(name="psbig", bufs=2, space="PSUM") as psbig,
            tc.tile_pool(name="psacc", bufs=1, space="PSUM") as psacc,
            tc.tile_pool(name="junka", bufs=4) as junka,
            tc.tile_pool(name="junkc", bufs=4) as junkc,
            tc.tile_pool(name="small", bufs=4) as small,
            tc.tile_pool(name="emb", bufs=4) as emb,
        ):
            # ---- load inputs
            emT_t = sb.tile([D, B], f32)
            emTA_t = sb.tile([D, AH], f32)
            rows_t = sb.tile([1, 2 * B], f32)
            cols_t = sb.tile([AH, 13 + KP], f32)
            idsAr_t = sb.tile([1, AH], f32)
            nc.sync.dma_start(out=emT_t[:], in_=emT[:])
            nc.sync.dma_start(out=emTA_t[:], in_=emTA[:])
            nc.sync.dma_start(out=rows_t[:], in_=rows[:])
            nc.sync.dma_start(out=cols_t[:], in_=cols[:])
            nc.sync.dma_start(out=idsAr_t[:], in_=idsAr[:])
            colsA_t = cols_t[:, 0:2]
            idsAll_t = cols_t[:, 2:6]
            idxAll_t = cols_t[:, 6:10]
            kidx_t = cols_t[:, 10 : 10 + KP]
            parc_t = cols_t[:, 10 + KP : 11 + KP]
            rowb_t = cols_t[:, 11 + KP : 12 + KP]
            gcol_t = cols_t[0:64, 12 + KP : 13 + KP]

            ones128 = sb.tile([D, 1], f32)
            nc.vector.memset(ones128[:], 1.0)
            ones128b = sb.tile([D, 1], bf16)
            nc.vector.memset(ones128b[:], 1.0)
            ones1 = sb.tile([1, D], f32)
            nc.vector.memset(ones1[:], 1.0)
            ones_row = sb.tile([1, B], f32)
            nc.vector.memset(ones_row[:], 1.0)

            # ---- squared norms
            sq_sb = sb.tile([1, B], f32)
            sqa_sb = sb.tile([1, AH], f32)
            e2 = sb.tile([D, B], f32)
            nc.vector.tensor_mul(e2[:], emT_t[:], emT_t[:])
            ps_sq = psrow.tile([1, B], f32, tag="row")
            nc.tensor.matmul(ps_sq[:], ones128[:], e2[:], start=True, stop=True)
            nc.scalar.copy(sq_sb[:], ps_sq[:])
            e2a = sb.tile([D, AH], f32)
            nc.vector.tensor_mul(e2a[:], emTA_t[:], emTA_t[:])
            ps_sqa = psrow.tile([1, AH], f32, tag="row")
            nc.tensor.matmul(ps_sqa[:], ones128[:], e2a[:], start=True, stop=True)
            nc.scalar.copy(sqa_sb[:], ps_sqa[:])

            emTAm2 = sb.tile([D, AH], f32)
            nc.vector.tensor_scalar_mul(emTAm2[:], emTA_t[:], -2.0)

            # d2 rows for this core's anchors, full n range
            ps_d2 = psbig.tile([AH, B], f32, tag="big")
            nc.tensor.matmul(ps_d2[:], emTAm2[:], emT_t[:], start=True, stop=False)
            nc.tensor.matmul(ps_d2[:], sqa_sb[:], ones_row[:], start=False, stop=False)
            nc.tensor.matmul(ps_d2[:], ones1[:, 0:AH], sq_sb[:], start=False, stop=True)
            d2r = sb.tile([AH, B], f32)
            nc.vector.tensor_scalar_max(d2r[:], ps_d2[:], 0.0)
            dch = sb.tile([AH, B], f32)
            nc.scalar.activation(dch[:], d2r[:], AF.Sqrt)
            dchd_v = dchd[:].rearrange("(a b) one -> a (b one)", a=AH)
            nc.sync.dma_start(out=dchd_v, in_=dch[:])

            # id/idx broadcast rows (persist through the scatter loop)
            ps_ids = psbig.tile([AH, B], f32, tag="big")
            nc.tensor.matmul(ps_ids[:], ones1[:], rows_t[0:1, 0:B], start=True, stop=True)
            ps_idx = psbig.tile([AH, B], f32, tag="big")
            nc.tensor.matmul(
                ps_idx[:], ones1[:], rows_t[0:1, B : 2 * B], start=True, stop=True
            )
            # copy broadcasts to SBUF so DVE readers get 2x mode (PSUM src
            # is capped at 1x with a 120-cycle init)
            ids_sb = sb.tile([AH, B], f32)
            nc.scalar.copy(ids_sb[:], ps_ids[:])
            idx_sb = sb.tile([AH, B], f32)
            nc.scalar.copy(idx_sb[:], ps_idx[:])

            # dneg = d + BIG*same; group size cA per anchor
            s_full = sb.tile([AH, B], f32)
            nc.vector.tensor_scalar(
                out=s_full[:], in0=ids_sb[:], scalar1=colsA_t[:, 0:1], scalar2=None,
                op0=OP.is_equal,
            )
            dneg = sb.tile([AH, B], f32)
            nc.vector.scalar_tensor_tensor(
                out=dneg[:], in0=s_full[:], scalar=BIG, in1=dch[:],
                op0=OP.mult, op1=OP.add,
            )
            dneg_b = sb.tile([AH, B], bf16)
            nc.vector.tensor_copy(dneg_b[:], dneg[:])
            cA = sb.tile([AH, 1], f32)
            nc.vector.reduce_sum(cA[:], s_full[:], axis=X)

            # ---- build member table M[g,k] = sum_i id-onehot * pos-onehot * i
            # via PE matmuls over 4 chunks of i, then ptab[a,:] = M[id_a,:]
            # via a second one-hot matmul. No indirect DMA needed.
            ps_mt = psrow.tile([64, KMAX], f32, tag="mt")
            for c4 in range(4):
                idc = idsAll_t[:, c4 : c4 + 1]
                ixc = idxAll_t[:, c4 : c4 + 1]
                # rank reduce only needs columns j < 128*(c4+1): larger j
                # exceed every i in this chunk, so jl would be 0 there
                W = AH * (c4 + 1)
                s4 = small.tile([AH, B], f32, tag="s4")
                nc.vector.tensor_scalar(
                    out=s4[:, 0:W], in0=ids_sb[:, 0:W], scalar1=idc, scalar2=None,
                    op0=OP.is_equal,
                )
                jl = small.tile([AH, B], f32, tag="jl")
                nc.vector.tensor_scalar(
                    out=jl[:, 0:W], in0=idx_sb[:, 0:W], scalar1=ixc, scalar2=None,
                    op0=OP.is_lt,
                )
                jm = small.tile([AH, B], f32, tag="jm")
                nc.vector.tensor_mul(jm[:, 0:W], jl[:, 0:W], s4[:, 0:W])
                r4 = small.tile([AH, 1], f32, tag="r4")
                jr = small.tile([AH, B], f32, tag="jr")
                nc.vector.tensor_scalar(
                    out=jr[:, 0:W], in0=jm[:, 0:W], scalar1=1.0, scalar2=None,
                    op0=OP.mult, op1=OP.add, accum_out=r4[:],
                )
                # perm: pos = R - 8*par + 16*[R < 8*par] -- rotates this
                # core's rank half to cols 0..7, parks the rest in 8..15
                w = small.tile([AH, 1], f32, tag="w")
                nc.vector.tensor_tensor(
                    out=w[:], in0=r4[:], in1=parc_t[:], op=OP.is_lt,
                )
                t16 = small.tile([AH, 1], f32, tag="t16")
                nc.vector.scalar_tensor_tensor(
                    out=t16[:], in0=w[:], scalar=16.0, in1=r4[:],
                    op0=OP.mult, op1=OP.add,
                )
                pos = small.tile([AH, 1], f32, tag="ps")
                nc.vector.tensor_sub(pos[:], t16[:], parc_t[:])
                # id one-hot [i, g] and (pos one-hot * index) [i, k]
                a4 = small.tile([AH, 64], f32, tag="a4")
                nc.vector.tensor_scalar(
                    out=a4[:], in0=idx_sb[:, 0:64], scalar1=idc, scalar2=None,
                    op0=OP.is_equal,
                )
                oh = small.tile([AH, KMAX], f32, tag="oh")
                nc.vector.tensor_scalar(
                    out=oh[:], in0=idx_sb[:, 0:KMAX], scalar1=pos[:, 0:1],
                    scalar2=None, op0=OP.is_equal,
                )
                bv = small.tile([AH, KMAX], f32, tag="bv")
                nc.vector.tensor_scalar(
                    out=bv[:], in0=oh[:], scalar1=ixc, scalar2=None, op0=OP.mult,
                )
                nc.tensor.matmul(
                    ps_mt[:], a4[:], bv[:], start=(c4 == 0), stop=(c4 == 3)
                )
            m_sb = sb.tile([64, KMAX], f32)
            nc.scalar.copy(m_sb[:], ps_mt[:])

            # ptab[a,:] = M[id_a,:] via one-hot over g (K=64 matmul)
            ps_ohb = psrow.tile([64, AH], f32, tag="ohb")
            nc.tensor.matmul(
                ps_ohb[:], ones1[0:1, 0:64], idsAr_t[:], start=True, stop=True
            )
            ohT = sb.tile([64, AH], f32)
            nc.vector.tensor_scalar(
                out=ohT[:], in0=ps_ohb[:], scalar1=gcol_t[:, 0:1], scalar2=None,
                op0=OP.is_equal,
            )
            ps_ptab = psrow.tile([AH, KMAX], f32, tag="ptab")
            nc.tensor.matmul(ps_ptab[:], ohT[:], m_sb[:], start=True, stop=True)
            pf = sb.tile([AH, KMAX], f32)
            nc.scalar.copy(pf[:], ps_ptab[:])
            selfm = sb.tile([AH, KP], f32)
            nc.vector.tensor_scalar(
                out=selfm[:], in0=pf[:, 0:KP], scalar1=colsA_t[:, 1:2], scalar2=None,
                op0=OP.is_equal,
            )
            kv = sb.tile([AH, KP], f32)
            nc.vector.tensor_scalar(
                out=kv[:], in0=kidx_t[:], scalar1=cA[:], scalar2=None, op0=OP.is_lt,
            )
            vm = sb.tile([AH, KP], f32)
            nc.vector.tensor_sub(vm[:], kv[:], selfm[:])

            # ---- fetch d[a, p] for every member column in one gather
            pfs = sb.tile([AH, KP], f32)
            nc.vector.tensor_scalar(
                out=pfs[:], in0=pf[:, 0:KP], scalar1=rowb_t[:, 0:1], scalar2=None,
                op0=OP.add,
            )
            offi = sb.tile([AH, KP], i32)
            nc.vector.tensor_copy(offi[:], pfs[:])
            xg = sb.tile([AH, KP], f32)
            xall = sb.tile([AH, KP], f32)

            # ---- main loop over KP member columns (gather -> mask -> ops
            # per column so the pipeline fills column by column)
            ps_relu = psacc.tile([1, B], f32)
            ps_cnt = psacc.tile([1, B], f32)

            for j in range(KP):
                nc.gpsimd.indirect_dma_start(
                    out=xg[:, j : j + 1], out_offset=None, in_=dchd[:],
                    in_offset=IOA(ap=offi[:, j : j + 1], axis=0),
                )
                djm = small.tile([AH, 1], f32, tag="djm")
                nc.vector.tensor_scalar_add(djm[:], xg[:, j : j + 1], MARGIN)
                nc.vector.tensor_mul(xall[:, j : j + 1], djm[:], vm[:, j : j + 1])
                xj = xall[:, j : j + 1]
                t = junka.tile([AH, B], bf16)
                nc.scalar.activation(t[:], dneg[:], AF.Relu, bias=xj[:], scale=-1.0)
                nc.tensor.matmul(
                    ps_relu[:], ones128b[:], t[:],
                    start=(j == 0), stop=(j == KP - 1),
                )
                g = junkc.tile([AH, B], bf16)
                nc.vector.tensor_scalar(
                    out=g[:], in0=dneg_b[:], scalar1=xj[:], scalar2=None, op0=OP.is_lt,
                )
                nc.tensor.matmul(
                    ps_cnt[:], ones128b[:], g[:],
                    start=(j == 0), stop=(j == KP - 1),
                )

            # ---- final
            res = sb.tile([1, 2], f32)
            nc.vector.reduce_sum(res[:, 0:1], ps_relu[:], axis=X)
            nc.vector.reduce_sum(res[:, 1:2], ps_cnt[:], axis=X)
            nc.sync.dma_start(out=out[:], in_=res[:])

    return nc


def _legalize_waits(bir: bytes) -> bytes:
    """walrus codegen in this toolchain allows only one sync-wait per
    instruction; split extra waits into standalone EventSemaphore insts."""
    import json

    m = json.loads(bir)
    for fn in m["functions"]:
        for bb in fn["blocks"]:
            new = []
            for inst in bb["instructions"]:
                si = inst.get("sync_info")
                if si and si.get("on_wait") and len(si["on_wait"]) > 1:
                    waits = si["on_wait"]
                    for j, w in enumerate(waits[:-1]):
                        new.append(
                            {
                                "engine": inst["engine"],
                                "ins": [],
                                "outs": [],
                                "name": f"{inst['name']}-w{j}",
                                "opcode": "EventSemaphore",
                                "sync_info": {"on_update": [], "on_wait": [w]},
                            }
                        )
                    si["on_wait"] = [waits[-1]]
                new.append(inst)
            bb["instructions"] = new
    return json.dumps(m).encode()


def _get_nc():
    if "nc" not in _CACHE:
        nc = _build_bass()
        orig = nc.to_json_bytes
        nc.to_json_bytes = lambda: _legalize_waits(orig())
        _CACHE["nc"] = nc
    return _CACHE["nc"]


def make_in_maps(embs: np.ndarray, idtys: np.ndarray):
    embs = np.ascontiguousarray(np.asarray(embs, dtype=np.float32))
    emT = np.ascontiguousarray(embs.T)  # [D, B]
    ids = np.asarray(idtys).astype(np.float32)
    idx = np.arange(B, dtype=np.float32)
    in_maps = []
    for c in range(NCORES):
        a0 = (c // 2) * AH
        par = c % 2
        rows = np.concatenate([ids, idx])[None, :]
        kcol = (np.arange(KP, dtype=np.float32) + 8.0 * par)[None, :]
        gc = np.zeros((AH, 1), dtype=np.float32)
        gc[:64, 0] = np.arange(64, dtype=np.float32)
        cols = np.concatenate(
            [
                ids[a0 : a0 + AH].reshape(AH, 1),
                idx[a0 : a0 + AH].reshape(AH, 1),
                ids.reshape(4, AH).T,
                idx.reshape(4, AH).T,
                np.repeat(kcol, AH, axis=0),
                np.full((AH, 1), 8.0 * par, dtype=np.float32),
                (np.arange(AH, dtype=np.float32) * B).reshape(AH, 1),
                gc,
            ],
            axis=1,
        ).astype(np.float32)
        in_maps.append(
            {
                "emT": emT,
                "emTA": np.ascontiguousarray(emT[:, a0 : a0 + AH]),
                "rows": np.ascontiguousarray(rows.astype(np.float32)),
                "cols": np.ascontiguousarray(cols),
                "idsAr": np.ascontiguousarray(ids[a0 : a0 + AH][None, :]),
            }
        )
    return in_maps


def combine(results):
    total = 0.0
    count = 0.0
    for r in results:
        o = np.asarray(r["out"], dtype=np.float64)
        total += o[0, 0]
        count += o[0, 1]
    loss = np.float32(total / (count + 1e-16))
    return np.array(loss, dtype=np.float32)


def kernel(embs: np.ndarray, idtys: np.ndarray) -> np.ndarray:
    from concourse import bass_utils

    nc = _get_nc()
    in_maps = make_in_maps(np.asarray(embs), np.asarray(idtys))
    res = bass_utils.run_bass_kernel_spmd(nc, in_maps, list(range(NCORES)))
    return combine(res.results)



# revision 8
# speedup vs baseline: 1.3549x; 1.3549x over previous
"""BatchAllTripletLoss on 8 Trainium2 NeuronCores (sorted-layout version).

Contract: kernel(**inputs) takes the FULL inputs (embs [512,128] f32,
idtys [512] int64) and returns the FULL output (scalar f32 loss).

Math: d = pairwise euclidean distances [512,512];
  loss = sum_{a,p,n} relu(d[a,p]-d[a,n]+margin)*mask / (num_pos + eps)
The mask factorizes as pos[a,p]*neg[a,n] (pos: same id, p!=a; neg:
different id). All index work happens on the host: samples are sorted
by id so each group is contiguous, and per core the sample order is
rotated so its 128 anchors sit at positions AOFF..AOFF+127 and every
anchor's group lies inside positions [0, 256).

Per-core device program (core c: anchor block b=c//2, parity par=c%2,
handling positive ranks {par, par+2, ...} = KP columns):
 1. d rows for the 128 anchors via PE matmul; sq[n] folded in as a
    rank-1 matmul, sq[a] + relu clamp fused into one DVE op; ACT sqrt.
 2. ndneg = -(d + BIG*same) in bf16 (same-mask is a host input; BIG
    pushes same-id negatives out of every relu/count).
 3. x[a,j] = (d[a, p_j(a)] + margin)*valid via a masked-transpose
    selection: dm = d * same (cols 0..255), PE-transpose the two
    128-wide chunks, matmul against a host-built rank-one-hot E.
    No DRAM round trip, no indirect DMA, no device argsort.
 4. Main loop over KP rank columns, all on DVE with fused row-accum:
    relu sum:  max(ndneg + x_j, 0)  -> accum racc[:, j]
    count:     (ndneg > -x_j)       -> accum cacc[:, j]
 5. Free-axis reduce + one ones-matmul -> out [1,2] = (sum, count);
    host sums cores and divides.
"""

import numpy as np

B = 512
D = 128
NCORES = 8
AH = 128          # anchors per core
AOFF = 16         # rotated position of the first anchor
MARGIN = 0.2
BIG = 1.0e6

_CACHE = {}


def _build_bass(KP):
    import concourse.bass as bass
    import concourse.tile as tile
    from concourse import mybir

    f32 = mybir.dt.float32
    bf16 = mybir.dt.bfloat16
    AF = mybir.ActivationFunctionType
    OP = mybir.AluOpType
    X = mybir.AxisListType.X

    nc = bass.Bass()

    emT = nc.dram_tensor("emT", [D, B], f32, kind="ExternalInput")
    msk = nc.dram_tensor("msk", [AH, B], f32, kind="ExternalInput")
    aux = nc.dram_tensor("aux", [AH, D + 3 * KP], f32, kind="ExternalInput")
    out = nc.dram_tensor("out", [1, 3], f32, kind="ExternalOutput")

    with tile.TileContext(nc) as tc:
        with (
            tc.tile_pool(name="sb", bufs=1) as sb,
            tc.tile_pool(name="junk", bufs=2) as junk,
            tc.tile_pool(name="psd", bufs=1, space="PSUM") as psd,
            tc.tile_pool(name="pst", bufs=1, space="PSUM") as pst,
            tc.tile_pool(name="pss", bufs=1, space="PSUM") as pss,
        ):
            emT_t = sb.tile([D, B], f32)
            msk_t = sb.tile([AH, B], f32)
            aux_t = sb.tile([AH, D + 3 * KP], f32)
            nc.sync.dma_start(out=emT_t[:], in_=emT[:])
            nc.sync.dma_start(out=msk_t[:], in_=msk[:])
            nc.sync.dma_start(out=aux_t[:], in_=aux[:])
            ident_t = aux_t[:, 0:D]
            E0_t = aux_t[:, D : D + KP]
            E1_t = aux_t[:, D + KP : D + 2 * KP]
            vm_t = aux_t[:, D + 2 * KP : D + 3 * KP]

            ones_c = sb.tile([D, 1], f32)
            nc.vector.memset(ones_c[:], 1.0)
            ones_r = sb.tile([1, D], f32)
            nc.vector.memset(ones_r[:], 1.0)
            one_1 = sb.tile([1, 1], f32)
            nc.vector.memset(one_1[:], 1.0)

            # ---- squared norms: sq[n] row, then anchor column sqa
            e2 = sb.tile([D, B], f32)
            nc.vector.tensor_mul(e2[:], emT_t[:], emT_t[:])
            ps_sq = pss.tile([1, B], f32, tag="sq")
            nc.tensor.matmul(ps_sq[:], ones_c[:], e2[:], start=True, stop=True)
            sq_sb = sb.tile([1, B], f32)
            nc.scalar.copy(sq_sb[:], ps_sq[:])
            ps_sqa = pss.tile([AH, 1], f32, tag="sqa")
            nc.tensor.matmul(
                ps_sqa[:], sq_sb[0:1, AOFF : AOFF + AH], one_1[:],
                start=True, stop=True,
            )
            sqa_sb = sb.tile([AH, 1], f32)
            nc.scalar.copy(sqa_sb[:], ps_sqa[:])

            # ---- d2 rows = sq[n] - 2 A^T E  (sq[a] + clamp fused below)
            emTAm2 = sb.tile([D, AH], f32)
            nc.vector.tensor_scalar_mul(emTAm2[:], emT_t[:, AOFF : AOFF + AH], -2.0)
            ps_d2 = psd.tile([AH, B], f32, tag="d2")
            nc.tensor.matmul(ps_d2[:], ones_r[:], sq_sb[:], start=True, stop=False)
            nc.tensor.matmul(ps_d2[:], emTAm2[:], emT_t[:], start=False, stop=True)
            d2c = sb.tile([AH, B], f32)
            nc.vector.tensor_scalar(
                out=d2c[:], in0=ps_d2[:], scalar1=sqa_sb[:], scalar2=0.0,
                op0=OP.add, op1=OP.max,
            )
            dch = sb.tile([AH, B], f32)
            nc.scalar.activation(dch[:], d2c[:], AF.Sqrt)

            # ---- x[a,j] via masked transpose + rank-selection matmul
            dm = sb.tile([AH, 256], f32)
            nc.vector.tensor_mul(dm[:], dch[:, 0:256], msk_t[:, 0:256])
            tp0 = pst.tile([128, 128], f32, tag="tp0")
            nc.tensor.transpose(tp0[:], dm[:, 0:128], ident_t[:])
            tp1 = pst.tile([128, 128], f32, tag="tp1")
            nc.tensor.transpose(tp1[:], dm[:, 128:256], ident_t[:])
            tp0_sb = sb.tile([128, 128], f32)
            nc.scalar.copy(tp0_sb[:], tp0[:])
            tp1_sb = sb.tile([128, 128], f32)
            nc.scalar.copy(tp1_sb[:], tp1[:])
            ps_xsel = pss.tile([AH, KP], f32, tag="xsel")
            nc.tensor.matmul(ps_xsel[:], tp0_sb[:], E0_t[:], start=True, stop=False)
            nc.tensor.matmul(ps_xsel[:], tp1_sb[:], E1_t[:], start=False, stop=True)
            xall = sb.tile([AH, KP], f32)
            nc.vector.scalar_tensor_tensor(
                out=xall[:], in0=ps_xsel[:], scalar=MARGIN, in1=vm_t[:],
                op0=OP.add, op1=OP.mult,
            )
            xneg = sb.tile([AH, KP], f32)
            nc.vector.tensor_scalar_mul(xneg[:], xall[:], -1.0)

            # ---- ndneg = -(d + BIG*same) in bf16
            ndneg_b = sb.tile([AH, B], bf16)
            nc.vector.scalar_tensor_tensor(
                out=ndneg_b[:], in0=msk_t[:], scalar=-BIG, in1=dch[:],
                op0=OP.mult, op1=OP.subtract,
            )

            # ---- main loop: KP rank columns, all DVE with fused row-accum
            # relu-sum identity: sum_n max(ndneg+x, 0) = sum_n max(ndneg, -x)
            # + B*x; the B*x correction is applied via rc[:,2] on the host.
            racc = sb.tile([AH, KP], f32)
            cacc = sb.tile([AH, KP], f32)
            for j in range(KP):
                t = junk.tile([AH, B], bf16, tag="t")
                nc.vector.tensor_scalar(
                    out=t[:], in0=ndneg_b[:], scalar1=xneg[:, j : j + 1],
                    scalar2=None, op0=OP.max, op1=OP.add,
                    accum_out=racc[:, j : j + 1],
                )
                g = junk.tile([AH, B], bf16, tag="g")
                nc.vector.tensor_scalar(
                    out=g[:], in0=ndneg_b[:], scalar1=xneg[:, j : j + 1],
                    scalar2=None, op0=OP.is_gt, op1=OP.add,
                    accum_out=cacc[:, j : j + 1],
                )

            # ---- final reduce: free-axis sums, one partition matmul, DMA
            rc = sb.tile([AH, 3], f32)
            nc.vector.reduce_sum(rc[:, 0:1], racc[:], axis=X)
            nc.vector.reduce_sum(rc[:, 1:2], cacc[:], axis=X)
            nc.vector.reduce_sum(rc[:, 2:3], xall[:], axis=X)
            ps_out = pss.tile([1, 3], f32, tag="out")
            nc.tensor.matmul(ps_out[:], ones_c[:], rc[:], start=True, stop=True)
            res = sb.tile([1, 3], f32)
            nc.scalar.copy(res[:], ps_out[:])
            nc.sync.dma_start(out=out[:], in_=res[:])

    return nc


def _legalize_waits(bir: bytes) -> bytes:
    """walrus codegen in this toolchain allows only one sync-wait per
    instruction; split extra waits into standalone EventSemaphore insts."""
    import json

    m = json.loads(bir)
    for fn in m["functions"]:
        for bb in fn["blocks"]:
            new = []
            for inst in bb["instructions"]:
                si = inst.get("sync_info")
                if si and si.get("on_wait") and len(si["on_wait"]) > 1:
                    waits = si["on_wait"]
                    for j, w in enumerate(waits[:-1]):
                        new.append(
                            {
                                "engine": inst["engine"],
                                "ins": [],
                                "outs": [],
                                "name": f"{inst['name']}-w{j}",
                                "opcode": "EventSemaphore",
                                "sync_info": {"on_update": [], "on_wait": [w]},
                            }
                        )
                    si["on_wait"] = [waits[-1]]
                new.append(inst)
            bb["instructions"] = new
    return json.dumps(m).encode()


def _get_nc(KP):
    key = ("nc", KP)
    if key not in _CACHE:
        nc = _build_bass(KP)
        orig = nc.to_json_bytes
        nc.to_json_bytes = lambda: _legalize_waits(orig())
        _CACHE[key] = nc
    return _CACHE[key]


def _prep(idtys):
    """Host-side index work: stable sort by id, group geometry."""
    ids = np.asarray(idtys).astype(np.int64).reshape(B)
    order = np.argsort(ids, kind="stable")
    ids_sorted = ids[order]
    g_start = np.zeros(B, np.int64)
    g_size = np.zeros(B, np.int64)
    _, starts, counts = np.unique(ids_sorted, return_index=True, return_counts=True)
    for s, c in zip(starts, counts):
        g_start[s : s + c] = s
        g_size[s : s + c] = c
    rank_sorted = np.arange(B) - g_start
    smax = int(counts.max())
    return order, ids_sorted, rank_sorted, g_size, smax


def make_in_maps(embs: np.ndarray, idtys: np.ndarray):
    embs = np.ascontiguousarray(np.asarray(embs, dtype=np.float32))
    order, ids_sorted, rank_sorted, g_size, smax = _prep(idtys)
    KP = max((smax + 1) // 2, 1)
    ident = np.eye(D, dtype=np.float32)
    idx = np.arange(B)
    in_maps = []
    for c in range(NCORES):
        b, par = c // 2, c % 2
        spos = (idx - AOFF + 128 * b) % B   # sorted position at rot position i
        rot = order[spos]                   # original sample at rot position i
        ids_rot = ids_sorted[spos]
        rank_rot = rank_sorted[spos]
        size_rot = g_size[spos]
        emT = np.ascontiguousarray(embs[rot].T)            # [D, B]
        a_sl = slice(AOFF, AOFF + AH)
        mask = (ids_rot[a_sl][:, None] == ids_rot[None, :]).astype(np.float32)
        E = np.zeros((256, KP), np.float32)
        r256 = rank_rot[:256]
        sel = (r256 % 2 == par) & (r256 // 2 < KP)
        E[np.nonzero(sel)[0], r256[sel] // 2] = 1.0
        ra, sa = rank_rot[a_sl], size_rot[a_sl]
        rk = 2 * np.arange(KP)[None, :] + par               # [1, KP]
        vm = ((rk < sa[:, None]) & (rk != ra[:, None])).astype(np.float32)
        auxm = np.concatenate([ident, E[:128], E[128:256], vm], axis=1)
        in_maps.append(
            {
                "emT": emT,
                "msk": np.ascontiguousarray(mask),
                "aux": np.ascontiguousarray(auxm.astype(np.float32)),
            }
        )
    return in_maps, KP


def combine(results):
    total = 0.0
    count = 0.0
    for r in results:
        o = np.asarray(r["out"], dtype=np.float64)
        total += o[0, 0] + B * o[0, 2]
        count += o[0, 1]
    loss = np.float32(total / (count + 1e-16))
    return np.array(loss, dtype=np.float32)


def kernel(embs: np.ndarray, idtys: np.ndarray) -> np.ndarray:
    from concourse import bass_utils

    in_maps, KP = make_in_maps(np.asarray(embs), np.asarray(idtys))
    nc = _get_nc(KP)
    res = bass_utils.run_bass_kernel_spmd(nc, in_maps, list(range(NCORES)))
    return combine(res.results)


# revision 9
# speedup vs baseline: 1.8855x; 1.3916x over previous
"""BatchAllTripletLoss on 8 Trainium2 NeuronCores (sorted-layout version).

Contract: kernel(**inputs) takes the FULL inputs (embs [512,128] f32,
idtys [512] int64) and returns the FULL output (scalar f32 loss).

Math: d = pairwise euclidean distances [512,512];
  loss = sum_{a,p,n} relu(d[a,p]-d[a,n]+margin)*mask / (num_pos + eps)
The mask factorizes as pos[a,p]*neg[a,n] (pos: same id, p!=a; neg:
different id). All index work happens on the host: samples are sorted
by id so each group is contiguous, and per core the sample order is
rotated so its 128 anchors sit at positions AOFF..AOFF+127 and every
anchor's group lies inside positions [0, 256).

Per-core device program (core c: anchor block b=c//2, parity par=c%2,
handling positive ranks {par, par+2, ...} = KP columns):
 1. d2 rows for the 128 anchors via three bf16 PE matmuls (-2*dot,
    ones@e2 for sq[n], rank-1 sq[n] broadcast); sq[a] comes from a DVE
    square-accumulate over a host-transposed anchor tile; sq[a]-add +
    relu clamp fused in one DVE op; ACT sqrt -> d.
 2. ndneg = -(d + BIG*same) in bf16 (same-mask is a host input; BIG
    pushes same-id negatives out of every relu/count).
 3. x2[a,j] = d2[a, p_j(a)] selected in SQUARED space (overlaps the
    big sqrt): dm2 = d2 * same (cols 0..255), PE-transpose the two
    128-wide chunks, bf16 matmul against a host-built rank-one-hot E,
    then a tiny sqrt + margin/valid fixup. No DRAM round trip, no
    indirect DMA, no device argsort.
 4. Main loop over KP rank columns, ACT and DVE in parallel:
    relu sum:  ACT Relu(ndneg + x_j) with accum_out -> racc[:, j]
    count:     DVE (ndneg > -x_j) accum           -> cacc[:, j]
 5. Free-axis reduce + one ones-matmul -> out [1,2] = (sum, count);
    host sums cores and divides.
"""

import numpy as np

B = 512
D = 128
NCORES = 8
AH = 128          # anchors per core
AOFF = 16         # rotated position of the first anchor
MARGIN = 0.2
BIG = 1.0e6

_CACHE = {}


def _build_bass(KP):
    import concourse.bass as bass
    import concourse.tile as tile
    from concourse import mybir

    f32 = mybir.dt.float32
    bf16 = mybir.dt.bfloat16
    AF = mybir.ActivationFunctionType
    OP = mybir.AluOpType
    X = mybir.AxisListType.X

    nc = bass.Bass()

    emT = nc.dram_tensor("emT", [D, B], f32, kind="ExternalInput")
    msk = nc.dram_tensor("msk", [AH, B], f32, kind="ExternalInput")
    aux = nc.dram_tensor("aux", [AH, D + KP], f32, kind="ExternalInput")
    emA = nc.dram_tensor("emA", [AH, D], f32, kind="ExternalInput")
    eb = nc.dram_tensor("eb", [AH, 2 * KP], bf16, kind="ExternalInput")
    out = nc.dram_tensor("out", [1, 2], f32, kind="ExternalOutput")

    with tile.TileContext(nc) as tc:
        with (
            tc.tile_pool(name="sb", bufs=1) as sb,
            tc.tile_pool(name="junk", bufs=2) as junk,
            tc.tile_pool(name="psd", bufs=1, space="PSUM") as psd,
            tc.tile_pool(name="pst", bufs=1, space="PSUM") as pst,
            tc.tile_pool(name="pss", bufs=1, space="PSUM") as pss,
        ):
            emT_t = sb.tile([D, B], f32)
            msk_t = sb.tile([AH, B], f32)
            aux_t = sb.tile([AH, D + KP], f32)
            emA_t = sb.tile([AH, D], f32)
            eb_t = sb.tile([AH, 2 * KP], bf16)
            nc.sync.dma_start(out=emT_t[:], in_=emT[:])
            nc.scalar.dma_start(out=msk_t[:], in_=msk[:])
            nc.sync.dma_start(out=emA_t[:], in_=emA[:])
            nc.scalar.dma_start(out=aux_t[:], in_=aux[:])
            nc.sync.dma_start(out=eb_t[:], in_=eb[:])
            ident_t = aux_t[:, 0:D]
            vm_t = aux_t[:, D : D + KP]
            E0_t = eb_t[:, 0:KP]
            E1_t = eb_t[:, KP : 2 * KP]

            ones_cb = sb.tile([D, 1], bf16)
            nc.vector.memset(ones_cb[:], 1.0)
            ones_rb = sb.tile([1, D], bf16)
            nc.vector.memset(ones_rb[:], 1.0)
            ones_cf = sb.tile([D, 1], f32)
            nc.vector.memset(ones_cf[:], 1.0)

            # ---- bf16 casts and sq[a] (DVE square-accumulate on emA)
            emT_b = sb.tile([D, B], bf16)
            nc.vector.tensor_copy(emT_b[:], emT_t[:])
            emTAm2 = sb.tile([D, AH], bf16)
            nc.vector.tensor_scalar_mul(emTAm2[:], emT_b[:, AOFF : AOFF + AH], -2.0)
            sqa_j = junk.tile([AH, D], bf16, tag="sqa")
            sqa_sb = sb.tile([AH, 1], f32)
            nc.vector.scalar_tensor_tensor(
                out=sqa_j[:], in0=emA_t[:], scalar=1.0, in1=emA_t[:],
                op0=OP.mult, op1=OP.mult, accum_out=sqa_sb[:],
            )
            e2b = sb.tile([D, B], bf16)
            nc.vector.tensor_mul(e2b[:], emT_b[:], emT_b[:])

            # ---- d2 rows = -2 A^T E + sq[n]  (sq[a] + clamp fused below)
            ps_d2 = psd.tile([AH, B], f32, tag="d2")
            nc.tensor.matmul(ps_d2[:], emTAm2[:], emT_b[:], start=True, stop=False)
            ps_sq = pss.tile([1, B], f32, tag="sq")
            nc.tensor.matmul(ps_sq[:], ones_cb[:], e2b[:], start=True, stop=True)
            sq_b = sb.tile([1, B], bf16)
            nc.scalar.copy(sq_b[:], ps_sq[:])
            nc.tensor.matmul(ps_d2[:], ones_rb[:], sq_b[:], start=False, stop=True)
            d2c = sb.tile([AH, B], f32)
            nc.vector.tensor_scalar(
                out=d2c[:], in0=ps_d2[:], scalar1=sqa_sb[:], scalar2=0.0,
                op0=OP.add, op1=OP.max,
            )
            dch = sb.tile([AH, B], f32)
            nc.scalar.activation(dch[:], d2c[:], AF.Sqrt)

            # ---- x2[a,j]: masked transpose + rank-selection in squared space
            dm2 = sb.tile([AH, 256], f32)
            nc.vector.tensor_mul(dm2[:], d2c[:, 0:256], msk_t[:, 0:256])
            tp0 = pst.tile([128, 128], f32, tag="tp0")
            nc.tensor.transpose(tp0[:], dm2[:, 0:128], ident_t[:])
            tp1 = pst.tile([128, 128], f32, tag="tp1")
            nc.tensor.transpose(tp1[:], dm2[:, 128:256], ident_t[:])
            tp0_sb = sb.tile([128, 128], bf16)
            nc.scalar.copy(tp0_sb[:], tp0[:])
            tp1_sb = sb.tile([128, 128], bf16)
            nc.scalar.copy(tp1_sb[:], tp1[:])
            ps_xsel = pss.tile([AH, KP], f32, tag="xsel")
            nc.tensor.matmul(ps_xsel[:], tp0_sb[:], E0_t[:], start=True, stop=False)
            nc.tensor.matmul(ps_xsel[:], tp1_sb[:], E1_t[:], start=False, stop=True)
            xsq = sb.tile([AH, KP], f32)
            nc.scalar.activation(xsq[:], ps_xsel[:], AF.Sqrt)
            xall = sb.tile([AH, KP], f32)
            nc.vector.scalar_tensor_tensor(
                out=xall[:], in0=xsq[:], scalar=MARGIN, in1=vm_t[:],
                op0=OP.add, op1=OP.mult,
            )
            xneg = sb.tile([AH, KP], f32)
            nc.vector.tensor_scalar_mul(xneg[:], xall[:], -1.0)

            # ---- ndneg = -(d + BIG*same) in bf16
            ndneg_b = sb.tile([AH, B], bf16)
            nc.vector.scalar_tensor_tensor(
                out=ndneg_b[:], in0=msk_t[:], scalar=-BIG, in1=dch[:],
                op0=OP.mult, op1=OP.subtract,
            )

            # ---- main loop: ACT relu-sum and DVE count, in parallel
            racc = sb.tile([AH, KP], f32)
            cacc = sb.tile([AH, KP], f32)
            for j in range(KP):
                t = junk.tile([AH, B], bf16, tag="t")
                nc.scalar.activation(
                    t[:], ndneg_b[:], AF.Relu, bias=xall[:, j : j + 1],
                    scale=1.0, accum_out=racc[:, j : j + 1],
                )
                g = junk.tile([AH, B], bf16, tag="g")
                nc.vector.tensor_scalar(
                    out=g[:], in0=ndneg_b[:], scalar1=xneg[:, j : j + 1],
                    scalar2=None, op0=OP.is_gt, op1=OP.add,
                    accum_out=cacc[:, j : j + 1],
                )

            # ---- final reduce: free-axis sums, one partition matmul, DMA
            rc = sb.tile([AH, 2], f32)
            nc.vector.reduce_sum(rc[:, 0:1], racc[:], axis=X)
            nc.vector.reduce_sum(rc[:, 1:2], cacc[:], axis=X)
            ps_out = pss.tile([1, 2], f32, tag="out")
            nc.tensor.matmul(ps_out[:], ones_cf[:], rc[:], start=True, stop=True)
            res = sb.tile([1, 2], f32)
            nc.scalar.copy(res[:], ps_out[:])
            nc.sync.dma_start(out=out[:], in_=res[:])

    return nc


def _legalize_waits(bir: bytes) -> bytes:
    """walrus codegen in this toolchain allows only one sync-wait per
    instruction; split extra waits into standalone EventSemaphore insts."""
    import json

    m = json.loads(bir)
    for fn in m["functions"]:
        for bb in fn["blocks"]:
            new = []
            for inst in bb["instructions"]:
                si = inst.get("sync_info")
                if si and si.get("on_wait") and len(si["on_wait"]) > 1:
                    waits = si["on_wait"]
                    for j, w in enumerate(waits[:-1]):
                        new.append(
                            {
                                "engine": inst["engine"],
                                "ins": [],
                                "outs": [],
                                "name": f"{inst['name']}-w{j}",
                                "opcode": "EventSemaphore",
                                "sync_info": {"on_update": [], "on_wait": [w]},
                            }
                        )
                    si["on_wait"] = [waits[-1]]
                new.append(inst)
            bb["instructions"] = new
    return json.dumps(m).encode()


def _get_nc(KP):
    key = ("nc", KP)
    if key not in _CACHE:
        nc = _build_bass(KP)
        orig = nc.to_json_bytes
        nc.to_json_bytes = lambda: _legalize_waits(orig())
        _CACHE[key] = nc
    return _CACHE[key]


def _prep(idtys):
    """Host-side index work: stable sort by id, group geometry."""
    ids = np.asarray(idtys).astype(np.int64).reshape(B)
    order = np.argsort(ids, kind="stable")
    ids_sorted = ids[order]
    g_start = np.zeros(B, np.int64)
    g_size = np.zeros(B, np.int64)
    _, starts, counts = np.unique(ids_sorted, return_index=True, return_counts=True)
    for s, c in zip(starts, counts):
        g_start[s : s + c] = s
        g_size[s : s + c] = c
    rank_sorted = np.arange(B) - g_start
    smax = int(counts.max())
    return order, ids_sorted, rank_sorted, g_size, smax


def make_in_maps(embs: np.ndarray, idtys: np.ndarray):
    import ml_dtypes

    embs = np.ascontiguousarray(np.asarray(embs, dtype=np.float32))
    order, ids_sorted, rank_sorted, g_size, smax = _prep(idtys)
    KP = max((smax + 1) // 2, 1)
    ident = np.eye(D, dtype=np.float32)
    idx = np.arange(B)
    in_maps = []
    for c in range(NCORES):
        b, par = c // 2, c % 2
        spos = (idx - AOFF + 128 * b) % B   # sorted position at rot position i
        rot = order[spos]                   # original sample at rot position i
        ids_rot = ids_sorted[spos]
        rank_rot = rank_sorted[spos]
        size_rot = g_size[spos]
        emT = np.ascontiguousarray(embs[rot].T)            # [D, B]
        a_sl = slice(AOFF, AOFF + AH)
        emA = np.ascontiguousarray(embs[rot[a_sl]])        # [AH, D]
        mask = (ids_rot[a_sl][:, None] == ids_rot[None, :]).astype(np.float32)
        E = np.zeros((256, KP), np.float32)
        r256 = rank_rot[:256]
        sel = (r256 % 2 == par) & (r256 // 2 < KP)
        E[np.nonzero(sel)[0], r256[sel] // 2] = 1.0
        ra, sa = rank_rot[a_sl], size_rot[a_sl]
        rk = 2 * np.arange(KP)[None, :] + par               # [1, KP]
        vm = ((rk < sa[:, None]) & (rk != ra[:, None])).astype(np.float32)
        auxm = np.concatenate([ident, vm], axis=1)
        ebm = np.concatenate([E[:128], E[128:256]], axis=1)
        in_maps.append(
            {
                "emT": emT,
                "msk": np.ascontiguousarray(mask),
                "aux": np.ascontiguousarray(auxm.astype(np.float32)),
                "emA": emA,
                "eb": np.ascontiguousarray(ebm.astype(ml_dtypes.bfloat16)),
            }
        )
    return in_maps, KP


def combine(results):
    total = 0.0
    count = 0.0
    for r in results:
        o = np.asarray(r["out"], dtype=np.float64)
        total += o[0, 0]
        count += o[0, 1]
    loss = np.float32(total / (count + 1e-16))
    return np.array(loss, dtype=np.float32)


def kernel(embs: np.ndarray, idtys: np.ndarray) -> np.ndarray:
    from concourse import bass_utils

    in_maps, KP = make_in_maps(np.asarray(embs), np.asarray(idtys))
    nc = _get_nc(KP)
    res = bass_utils.run_bass_kernel_spmd(nc, in_maps, list(range(NCORES)))
    return combine(res.results)
